# revision 9
# baseline (speedup 1.0000x reference)
"""Causal self-attention (B=4, T=2048, C=1024, H=16, D=64) on 8 trn2 NeuronCores.

Sharding: (batch, head-half). Core g owns batch b=g//2 and head-half
hh=g%2 -> heads [8*hh, 8*hh+8) = 4 head-pairs, 512 channels. Each core
computes out_partial[2048, 1024] = y_half @ W_proj[rows of its half];
the host sums the two partials per batch (row-parallel W_proj) + b_proj.
vs. pure head-parallel sharding this cuts per-core DMA 4x (x in, out out)
at identical PE cycle count.

All matmul operands are bf16: the PE streams bf16 at 1 cyc/elem vs ~2 for
fp32. PSUM accumulation stays fp32; the softmax normalizer chain stays fp32.

Device structure per core (S^T = K @ Q^T formulation):
  - S^T tiles are [k_tok, q_tok]: exp(S)*mask and P^T @ V need no on-chip
    transposes of P.
  - V^T tiles ("va", layout [1 | V_A^T] [1 | V_B^T]) are produced by the
    DMA xbar transpose, not the PE; the leading ones column makes the PV
    matmul emit the softmax normalizer l[q] on PSUM partition 0.
  - 1/l is computed on 32 DVE lanes via the 32x32 stream-transpose trick,
    then broadcast across partitions by GpSimd partition_broadcast (no PE
    involvement in normalization at all).
  - kt-loop is software-pipelined depth 1: S(kt+1) is issued before PV(kt)
    so the PE streams S while ScalarE works exp(kt).
  - QKV projection / V-transposes / output projection of neighboring tiles
    are interleaved between attention q-tiles to keep the PE dense.
"""

import numpy as np

B, T, C, H, D = 4, 2048, 1024, 16, 64
NCORES = 8
HPC = 8                  # heads per core
NHP = 4                  # head-pairs per core
CPC = HPC * D            # 512 channels per core
NC_CHUNKS = C // 128     # 8 contraction chunks of X^T
GC = 3 * CPC             # 1536 qkv-projection output cols per core
QW = 512                 # q-tile width (moving dim)
KW = 128                 # k-tile width (S^T partition dim)
NTT = T // QW            # 4 token tiles
SCALE = 1.0 / np.sqrt(D)

_CACHE = {}
LAST_RESULTS = None      # test harness reads exec_time_ns from here


def _build_bass():
    import concourse.bass as bass
    import concourse.mybir as mybir
    import concourse.tile as tile
    from concourse import bacc
    from concourse.masks import make_upper_triangular

    f32 = mybir.dt.float32
    bf16 = mybir.dt.bfloat16
    Exp = mybir.ActivationFunctionType.Exp

    nc = bacc.Bacc()
    xt = nc.dram_tensor("xt", [C, T], bf16, kind="ExternalInput")
    wg = nc.dram_tensor("wg", [C, GC], bf16, kind="ExternalInput")
    bg = nc.dram_tensor("bg", [GC], f32, kind="ExternalInput")
    wp = nc.dram_tensor("wp", [CPC, C], bf16, kind="ExternalInput")
    outp = nc.dram_tensor("outp", [T, C], f32, kind="ExternalOutput")

    with tile.TileContext(nc) as tc:
        with (
            tc.tile_pool(name="const", bufs=1) as cpool,
            tc.tile_pool(name="sb", bufs=2) as sb,
            tc.tile_pool(name="ps", bufs=2, space="PSUM") as ps,
        ):
            # ---- weights: wg on the gpsimd DMA ring, rest on sync, so the
            # first QKV matmuls have their operands ASAP ----
            wg_sb = []
            for ci in range(NC_CHUNKS):
                t_ = cpool.tile([128, GC], bf16, tag=f"wg{ci}", name="wgt")
                nc.gpsimd.dma_start(out=t_, in_=wg[ci * 128:(ci + 1) * 128, :])
                wg_sb.append(t_)
            # bias chunk o = grp*4 + hp lives in bias_sb[:, o]
            bias_sb = cpool.tile([128, 12], f32, tag="bias")
            nc.sync.dma_start(out=bias_sb, in_=bg.rearrange("(o p) -> p o", p=128))

            # ---- constants ----
            # mask[k, q] = 1.0 where q >= k else 0 (upper triangular incl diag)
            scratch2 = cpool.tile([128, 128], f32, tag="scratch2")
            make_upper_triangular(nc, scratch2, val=1.0, diag=True)
            mask = cpool.tile([128, 128], bf16, tag="mask")
            nc.vector.tensor_copy(mask, scratch2)
            # broadcast mask over the head axis (free-dim stride 0)
            mask2 = bass.AP(
                tensor=mask.tensor, offset=mask.offset,
                ap=[mask.ap[0], [0, 2], mask.ap[1]],
            )

            # ---- per-core persistent state ----
            # qkv_sb[grp][hp]: [128, T] bf16, rows 0:64 head 2hp, 64:128 head 2hp+1
            qkv_sb = [
                [cpool.tile([128, T], bf16, tag=f"{nm}{hp}", name=f"{nm}{hp}")
                 for hp in range(NHP)]
                for nm in ("q", "k", "v")
            ]
            vaug = [[None] * (T // KW) for _ in range(NHP)]

            def qkv_toktile(tt):
                """QKV projection for tokens [tt*QW, (tt+1)*QW)."""
                sl = slice(tt * QW, (tt + 1) * QW)
                xts = []
                for ci in range(NC_CHUNKS):
                    xtile = sb.tile([128, QW], bf16, tag="xt", bufs=16, name="xtile")
                    nc.sync.dma_start(out=xtile, in_=xt[ci * 128:(ci + 1) * 128, sl])
                    xts.append(xtile)
                for o in range(12):
                    grp, hp = divmod(o, NHP)
                    pq = ps.tile([128, QW], f32, tag="mm", name="pq")
                    for ci in range(NC_CHUNKS):
                        nc.tensor.matmul(
                            pq, wg_sb[ci][:, o * 128:(o + 1) * 128], xts[ci],
                            start=(ci == 0), stop=(ci == NC_CHUNKS - 1),
                        )
                    # bias-add doubles as the PSUM->SBUF+cast copy
                    nc.vector.tensor_scalar_add(
                        out=qkv_sb[grp][hp][:, sl], in0=pq,
                        scalar1=bias_sb[:, o:o + 1],
                    )

            def vtrans_toktile(tt):
                """V^T -> [1 | V_A^T | 1 | V_B^T] tiles via DMA xbar transpose."""
                for hp in range(NHP):
                    vt = qkv_sb[2][hp]
                    for kt in range(tt * 4, tt * 4 + 4):
                        # per-head stride 80 keeps each xbar-transpose dest
                        # 32B-aligned (65/72 corrupt the write)
                        va = cpool.tile([128, 2, 80], bf16, tag=f"va{hp}_{kt}",
                                        name="va")
                        nc.gpsimd.memset(va[:, :, D:D + 1], 1.0)
                        ksl = slice(kt * KW, (kt + 1) * KW)
                        nc.sync.dma_start_transpose(va[:, 0, 0:D], vt[0:64, ksl])
                        nc.sync.dma_start_transpose(va[:, 1, 0:D], vt[64:128, ksl])
                        vaug[hp][kt] = va

            def attention_qtile(qt, hp, yts):
                qt_sb, kt_sb = qkv_sb[0][hp], qkv_sb[1][hp]
                # y2 rows: 0..63 = y^T, 64 = l (from the ones column)
                y2 = ps.tile([D + 1, 2, QW], f32, tag="y", bufs=1, name="y2")
                nkt = (qt + 1) * (QW // KW)
                kdiag = qt * (QW // KW)      # first diagonal k-tile

                def s_mm(kt):
                    diag = kt >= kdiag
                    qoff = (kt - kdiag) * KW if diag else 0
                    w = QW - qoff
                    qsl = slice(qt * QW + qoff, (qt + 1) * QW)
                    ksl = slice(kt * KW, (kt + 1) * KW)
                    st = ps.tile([128, 2, QW], f32, tag="st", name="st")
                    nc.tensor.matmul(
                        st[:, 0, 0:w], kt_sb[0:64, ksl], qt_sb[0:64, qsl]
                    )
                    nc.tensor.matmul(
                        st[:, 1, 0:w], kt_sb[64:128, ksl], qt_sb[64:128, qsl]
                    )
                    p = sb.tile([128, 2, QW], bf16, tag="p", bufs=4, name="p")
                    nc.scalar.activation(
                        p[:, :, 0:w], st[:, :, 0:w], Exp, scale=SCALE
                    )
                    if diag:
                        nc.vector.tensor_mul(p[:, :, 0:KW], p[:, :, 0:KW], mask2)
                    return (p, qoff, w, kt)

                def pv_mm(t):
                    p, qoff, w, kt = t
                    va = vaug[hp][kt]
                    nc.tensor.matmul(
                        y2[:, 0, qoff:QW], va[:, 0, 0:D + 1], p[:, 0, 0:w],
                        start=(kt == 0), stop=(kt == nkt - 1),
                    )
                    nc.tensor.matmul(
                        y2[:, 1, qoff:QW], va[:, 1, 0:D + 1], p[:, 1, 0:w],
                        start=(kt == 0), stop=(kt == nkt - 1),
                    )

                # depth-1 pipeline: S(kt+1) issues before PV(kt) so the PE
                # streams S while ScalarE runs exp(kt)
                prev = s_mm(0)
                for kt in range(1, nkt):
                    cur = s_mm(kt)
                    pv_mm(prev)
                    prev = cur
                pv_mm(prev)

                # ---- normalize: y^T * broadcast(1/l) -> yt_sb [128, QW] ----
                ystage = sb.tile([128, 2, QW], f32, tag="ystage", name="ystage")
                nc.vector.tensor_copy(ystage[0:D + 1, :, :], y2[0:D + 1, :, :])
                # 1/l on 32 DVE lanes via 32x32 stream-transpose (l is row D)
                lrow = ystage.rearrange("p h q -> p (h q)")
                # cross-base stream transpose lands the chain on partitions
                # 0:32 so partition_broadcast (which reads the tile's
                # partition 0) sees 1/l on row 0 of rcf
                lt = sb.tile([32, 2 * QW], f32, tag="lt", name="lt")
                nc.vector.transpose(lt[0:32, :], lrow[D:D + 32, :])
                rt = sb.tile([32, 2 * QW], f32, tag="rt", name="rt")
                lt_v = lt[0:32, :].rearrange("p (j c) -> p j c", c=32)
                rt_v = rt[0:32, :].rearrange("p (j c) -> p j c", c=32)
                nc.vector.reciprocal(rt_v[:, :, 0:1], lt_v[:, :, 0:1])
                rcf = sb.tile([32, 2 * QW], f32, tag="rcf", name="rcf")
                nc.vector.transpose(rcf[0:32, :], rt[0:32, :])
                # broadcast 1/l across 64 partitions on GpSimd (PE-free)
                bc = sb.tile([64, 2, QW], f32, tag="bc", name="bc")
                nc.gpsimd.partition_broadcast(
                    bc.rearrange("p h q -> p (h q)"), rcf[0:1, :]
                )
                yt_sb = sb.tile([128, QW], bf16, tag="yt", bufs=8, name="yt_sb")
                nc.vector.tensor_mul(yt_sb[0:64, :], ystage[0:D, 0, :], bc[:, 0, :])
                nc.vector.tensor_mul(yt_sb[64:128, :], ystage[0:D, 1, :], bc[:, 1, :])
                yts[hp] = yt_sb

            def proj_qtile(qt, yts):
                for m in range(QW // 128):
                    osb = sb.tile([128, C], f32, tag="osb", bufs=3, name="osb")
                    for n in range(2):
                        pp = ps.tile([128, 512], f32, tag="mm", name="pp")
                        for hp in range(NHP):
                            nc.tensor.matmul(
                                pp, yts[hp][:, m * 128:(m + 1) * 128],
                                wp_sb[hp][:, n * 512:(n + 1) * 512],
                                start=(hp == 0), stop=(hp == NHP - 1),
                            )
                        if n == 0:
                            nc.scalar.copy(osb[:, 0:512], pp)
                        else:
                            nc.vector.tensor_copy(osb[:, 512:1024], pp)
                    row0 = qt * QW + m * 128
                    nc.gpsimd.dma_start(out=outp[row0:row0 + 128, :], in_=osb)

            # ---- software-pipelined schedule ----
            qkv_toktile(0)
            # wp only needed by the first proj (~40us in): DMA it after the
            # startup-critical loads
            wp_sb = []
            for hp in range(NHP):
                t_ = cpool.tile([128, C], bf16, tag=f"wp{hp}", name="wpt")
                nc.sync.dma_start(out=t_, in_=wp[hp * 128:(hp + 1) * 128, :])
                wp_sb.append(t_)
            vtrans_toktile(0)
            pending = []
            for qt in range(NTT):
                yts = {}
                for hp in range(NHP):
                    attention_qtile(qt, hp, yts)
                    if hp == 0 and qt + 1 < NTT:
                        qkv_toktile(qt + 1)
                    if hp == 1 and qt + 1 < NTT:
                        vtrans_toktile(qt + 1)
                    if hp == 2 and pending:
                        proj_qtile(*pending.pop())
                pending.append((qt, yts))
            proj_qtile(*pending.pop())

    nc.finalize()
    return nc


def _get_nc():
    if "nc" not in _CACHE:
        _CACHE["nc"] = _build_bass()
    return _CACHE["nc"]


def kernel(x, W_attn, b_attn, W_proj, b_proj):
    global LAST_RESULTS
    from concourse import bass_utils
    import ml_dtypes

    bf = ml_dtypes.bfloat16
    x = np.asarray(x, dtype=np.float32)
    W_attn = np.asarray(W_attn, dtype=np.float32)
    b_attn = np.asarray(b_attn, dtype=np.float32)
    W_proj = np.asarray(W_proj, dtype=np.float32)
    b_proj = np.asarray(b_proj, dtype=np.float32)

    in_maps = []
    for g in range(NCORES):
        b, hh = divmod(g, 2)
        cols = slice(hh * CPC, (hh + 1) * CPC)
        wg_g = np.ascontiguousarray(np.concatenate(
            [W_attn[:, cols], W_attn[:, C:][:, cols], W_attn[:, 2 * C:][:, cols]],
            axis=1,
        ).astype(bf))
        bg_g = np.ascontiguousarray(np.concatenate(
            [b_attn[cols], b_attn[C:][cols], b_attn[2 * C:][cols]]
        ))
        wp_g = np.ascontiguousarray(W_proj[cols, :].astype(bf))
        xt_g = np.ascontiguousarray(x[b].T.astype(bf))
        in_maps.append({"xt": xt_g, "wg": wg_g, "bg": bg_g, "wp": wp_g})

    nc = _get_nc()
    res = bass_utils.run_bass_kernel_spmd(nc, in_maps, core_ids=list(range(NCORES)))
    LAST_RESULTS = res

    out = np.empty((B, T, C), dtype=np.float32)
    for b in range(B):
        acc = res.results[2 * b]["outp"].astype(np.float64)
        acc += res.results[2 * b + 1]["outp"]
        acc += b_proj
        out[b] = acc.astype(np.float32)
    return out


# revision 10
# speedup vs baseline: 1.2652x; 1.2652x over previous
"""Causal self-attention (B=4, T=2048, C=1024, H=16, D=64) on 8 trn2 NeuronCores.

Sharding: (batch, head-half). Core g owns batch b=g//2 and head-half
hh=g%2 -> heads [8*hh, 8*hh+8) = 4 head-pairs, 512 channels. Each core
computes out_partial[2048, 1024] = y_half @ W_proj[rows of its half];
the host sums the two partials per batch (row-parallel W_proj) + b_proj.
vs. pure head-parallel sharding this cuts per-core DMA 4x (x in, out out)
at identical PE cycle count.

All matmul operands are bf16: the PE streams bf16 at 1 cyc/elem vs ~2 for
fp32. PSUM accumulation stays fp32; the softmax normalizer chain stays fp32.

Device structure per core (S^T = K @ Q^T formulation):
  - S^T tiles are [k_tok, q_tok]: exp(S)*mask and P^T @ V need no on-chip
    transposes of P.
  - V^T tiles ("va", layout [1 | V_A^T] [1 | V_B^T]) are produced by the
    DMA xbar transpose, not the PE; the leading ones column makes the PV
    matmul emit the softmax normalizer l[q] on PSUM partition 0.
  - 1/l is computed on 32 DVE lanes via the 32x32 stream-transpose trick,
    then broadcast across partitions by GpSimd partition_broadcast (no PE
    involvement in normalization at all).
  - kt-loop is software-pipelined depth 1: S(kt+1) is issued before PV(kt)
    so the PE streams S while ScalarE works exp(kt).
  - QKV projection / V-transposes / output projection of neighboring tiles
    are interleaved between attention q-tiles to keep the PE dense.
"""

import numpy as np

B, T, C, H, D = 4, 2048, 1024, 16, 64
NCORES = 8
HPC = 8                  # heads per core
NHP = 4                  # head-pairs per core
CPC = HPC * D            # 512 channels per core
NC_CHUNKS = C // 128     # 8 contraction chunks of X^T
GC = 3 * CPC             # 1536 qkv-projection output cols per core
QW = 512                 # q-tile width (moving dim)
KW = 128                 # k-tile width (S^T partition dim)
NTT = T // QW            # 4 token tiles
SCALE = 1.0 / np.sqrt(D)

_CACHE = {}
LAST_RESULTS = None      # test harness reads exec_time_ns from here


def _build_bass():
    import concourse.bass as bass
    import concourse.mybir as mybir
    import concourse.tile as tile
    from concourse import bacc
    from concourse.masks import make_upper_triangular

    f32 = mybir.dt.float32
    bf16 = mybir.dt.bfloat16
    Exp = mybir.ActivationFunctionType.Exp

    nc = bacc.Bacc()
    xt = nc.dram_tensor("xt", [C, T], bf16, kind="ExternalInput")
    wg = nc.dram_tensor("wg", [C, GC], bf16, kind="ExternalInput")
    bg = nc.dram_tensor("bg", [GC], f32, kind="ExternalInput")
    wp = nc.dram_tensor("wp", [CPC, C], bf16, kind="ExternalInput")
    outp = nc.dram_tensor("outp", [T, C], f32, kind="ExternalOutput")

    with tile.TileContext(nc) as tc:
        with (
            tc.tile_pool(name="const", bufs=1) as cpool,
            tc.tile_pool(name="sb", bufs=2) as sb,
            tc.tile_pool(name="ps", bufs=2, space="PSUM") as ps,
        ):
            # ---- weights: wg on the gpsimd DMA ring, rest on sync, so the
            # first QKV matmuls have their operands ASAP ----
            wg_sb = []
            for ci in range(NC_CHUNKS):
                t_ = cpool.tile([128, GC], bf16, tag=f"wg{ci}", name="wgt")
                nc.gpsimd.dma_start(out=t_, in_=wg[ci * 128:(ci + 1) * 128, :])
                wg_sb.append(t_)
            # bias chunk o = grp*4 + hp lives in bias_sb[:, o]
            bias_sb = cpool.tile([128, 12], f32, tag="bias")
            nc.sync.dma_start(out=bias_sb, in_=bg.rearrange("(o p) -> p o", p=128))

            # ---- constants ----
            scratch = cpool.tile([128, 128], f32, tag="scratch")
            from concourse.masks import make_identity
            make_identity(nc, scratch)
            identity = cpool.tile([128, 128], bf16, tag="ident")
            nc.vector.tensor_copy(identity, scratch)
            # mask[k, q] = 1.0 where q >= k else 0 (upper triangular incl diag)
            scratch2 = cpool.tile([128, 128], f32, tag="scratch2")
            make_upper_triangular(nc, scratch2, val=1.0, diag=True)
            mask = cpool.tile([128, 128], bf16, tag="mask")
            nc.vector.tensor_copy(mask, scratch2)
            # broadcast mask over the head axis (free-dim stride 0)
            mask2 = bass.AP(
                tensor=mask.tensor, offset=mask.offset,
                ap=[mask.ap[0], [0, 2], mask.ap[1]],
            )

            # ---- per-core persistent state ----
            # qkv_sb[grp][hp]: [128, T] bf16, rows 0:64 head 2hp, 64:128 head 2hp+1
            qkv_sb = [
                [cpool.tile([128, T], bf16, tag=f"{nm}{hp}", name=f"{nm}{hp}")
                 for hp in range(NHP)]
                for nm in ("q", "k", "v")
            ]
            vaug = [[None] * (T // KW) for _ in range(NHP)]

            def qkv_toktile(tt):
                """QKV projection for tokens [tt*QW, (tt+1)*QW)."""
                sl = slice(tt * QW, (tt + 1) * QW)
                xts = []
                for ci in range(NC_CHUNKS):
                    xtile = sb.tile([128, QW], bf16, tag="xt", bufs=16, name="xtile")
                    nc.sync.dma_start(out=xtile, in_=xt[ci * 128:(ci + 1) * 128, sl])
                    xts.append(xtile)
                for o in range(12):
                    grp, hp = divmod(o, NHP)
                    pq = ps.tile([128, QW], f32, tag="mm", name="pq")
                    for ci in range(NC_CHUNKS):
                        nc.tensor.matmul(
                            pq, wg_sb[ci][:, o * 128:(o + 1) * 128], xts[ci],
                            start=(ci == 0), stop=(ci == NC_CHUNKS - 1),
                        )
                    # bias-add doubles as the PSUM->SBUF+cast copy
                    nc.vector.tensor_scalar_add(
                        out=qkv_sb[grp][hp][:, sl], in0=pq,
                        scalar1=bias_sb[:, o:o + 1],
                    )

            def vtrans_toktile(tt):
                """V^T -> [V_A^T | 1 | V_B^T | 1] tiles via PE transpose."""
                for hp in range(NHP):
                    vt = qkv_sb[2][hp]
                    for kt in range(tt * 4, tt * 4 + 4):
                        ptr = ps.tile([128, 128], bf16, tag="mm",
                                      padded_shape=[128, 1024], name="ptr")
                        nc.tensor.transpose(
                            ptr, vt[:, kt * KW:(kt + 1) * KW], identity
                        )
                        va = cpool.tile([128, 2, D + 1], bf16, tag=f"va{hp}_{kt}",
                                        name="va")
                        nc.gpsimd.memset(va[:, :, D:D + 1], 1.0)
                        nc.vector.tensor_copy(
                            va[:, :, 0:D],
                            ptr[:, 0:2 * D].rearrange("p (h x) -> p h x", x=D),
                        )
                        vaug[hp][kt] = va

            def attention_qtile(qt, hp, yts):
                qt_sb, kt_sb = qkv_sb[0][hp], qkv_sb[1][hp]
                # y2 rows: 0..63 = y^T, 64 = l (from the ones column)
                y2 = ps.tile([D + 1, 2, QW], f32, tag="y", bufs=1, name="y2")
                nkt = (qt + 1) * (QW // KW)
                kdiag = qt * (QW // KW)      # first diagonal k-tile

                def s_mm(kt):
                    diag = kt >= kdiag
                    qoff = (kt - kdiag) * KW if diag else 0
                    w = QW - qoff
                    qsl = slice(qt * QW + qoff, (qt + 1) * QW)
                    ksl = slice(kt * KW, (kt + 1) * KW)
                    st = ps.tile([128, 2, QW], f32, tag="st", name="st")
                    nc.tensor.matmul(
                        st[:, 0, 0:w], kt_sb[0:64, ksl], qt_sb[0:64, qsl]
                    )
                    nc.tensor.matmul(
                        st[:, 1, 0:w], kt_sb[64:128, ksl], qt_sb[64:128, qsl]
                    )
                    p = sb.tile([128, 2, QW], bf16, tag="p", bufs=4, name="p")
                    nc.scalar.activation(
                        p[:, :, 0:w], st[:, :, 0:w], Exp, scale=SCALE
                    )
                    if diag:
                        nc.vector.tensor_mul(p[:, :, 0:KW], p[:, :, 0:KW], mask2)
                    return (p, qoff, w, kt)

                def pv_mm(t):
                    p, qoff, w, kt = t
                    va = vaug[hp][kt]
                    nc.tensor.matmul(
                        y2[:, 0, qoff:QW], va[:, 0, :], p[:, 0, 0:w],
                        start=(kt == 0), stop=(kt == nkt - 1),
                    )
                    nc.tensor.matmul(
                        y2[:, 1, qoff:QW], va[:, 1, :], p[:, 1, 0:w],
                        start=(kt == 0), stop=(kt == nkt - 1),
                    )

                # depth-1 pipeline: S(kt+1) issues before PV(kt) so the PE
                # streams S while ScalarE runs exp(kt)
                prev = s_mm(0)
                for kt in range(1, nkt):
                    cur = s_mm(kt)
                    pv_mm(prev)
                    prev = cur
                pv_mm(prev)

                # ---- normalize: y^T * broadcast(1/l) -> yt_sb [128, QW] ----
                ystage = sb.tile([128, 2, QW], f32, tag="ystage", name="ystage")
                nc.vector.tensor_copy(ystage[0:D + 1, :, :], y2[0:D + 1, :, :])
                # 1/l on 32 DVE lanes via 32x32 stream-transpose (l is row D)
                lrow = ystage.rearrange("p h q -> p (h q)")
                # cross-base stream transpose lands the chain on partitions
                # 0:32 so partition_broadcast (which reads the tile's
                # partition 0) sees 1/l on row 0 of rcf
                lt = sb.tile([32, 2 * QW], f32, tag="lt", name="lt")
                nc.vector.transpose(lt[0:32, :], lrow[D:D + 32, :])
                rt = sb.tile([32, 2 * QW], f32, tag="rt", name="rt")
                lt_v = lt[0:32, :].rearrange("p (j c) -> p j c", c=32)
                rt_v = rt[0:32, :].rearrange("p (j c) -> p j c", c=32)
                nc.vector.reciprocal(rt_v[:, :, 0:1], lt_v[:, :, 0:1])
                rcf = sb.tile([32, 2 * QW], f32, tag="rcf", name="rcf")
                nc.vector.transpose(rcf[0:32, :], rt[0:32, :])
                # broadcast 1/l across 64 partitions on GpSimd (PE-free)
                bc = sb.tile([64, 2, QW], f32, tag="bc", name="bc")
                nc.gpsimd.partition_broadcast(
                    bc.rearrange("p h q -> p (h q)"), rcf[0:1, :]
                )
                yt_sb = sb.tile([128, QW], bf16, tag="yt", bufs=8, name="yt_sb")
                nc.vector.tensor_mul(yt_sb[0:64, :], ystage[0:D, 0, :], bc[:, 0, :])
                nc.vector.tensor_mul(yt_sb[64:128, :], ystage[0:D, 1, :], bc[:, 1, :])
                yts[hp] = yt_sb

            def proj_qtile(qt, yts):
                for m in range(QW // 128):
                    osb = sb.tile([128, C], f32, tag="osb", bufs=3, name="osb")
                    for n in range(2):
                        pp = ps.tile([128, 512], f32, tag="mm", name="pp")
                        for hp in range(NHP):
                            nc.tensor.matmul(
                                pp, yts[hp][:, m * 128:(m + 1) * 128],
                                wp_sb[hp][:, n * 512:(n + 1) * 512],
                                start=(hp == 0), stop=(hp == NHP - 1),
                            )
                        if n == 0:
                            nc.scalar.copy(osb[:, 0:512], pp)
                        else:
                            nc.vector.tensor_copy(osb[:, 512:1024], pp)
                    row0 = qt * QW + m * 128
                    nc.gpsimd.dma_start(out=outp[row0:row0 + 128, :], in_=osb)

            # ---- software-pipelined schedule ----
            qkv_toktile(0)
            # wp only needed by the first proj (~40us in): DMA it after the
            # startup-critical loads
            wp_sb = []
            for hp in range(NHP):
                t_ = cpool.tile([128, C], bf16, tag=f"wp{hp}", name="wpt")
                nc.sync.dma_start(out=t_, in_=wp[hp * 128:(hp + 1) * 128, :])
                wp_sb.append(t_)
            vtrans_toktile(0)
            pending = []
            for qt in range(NTT):
                yts = {}
                for hp in range(NHP):
                    attention_qtile(qt, hp, yts)
                    if hp == 0 and qt + 1 < NTT:
                        qkv_toktile(qt + 1)
                    if hp == 1 and qt + 1 < NTT:
                        vtrans_toktile(qt + 1)
                    if hp == 2 and pending:
                        proj_qtile(*pending.pop())
                pending.append((qt, yts))
            proj_qtile(*pending.pop())

    nc.finalize()
    return nc


def _get_nc():
    if "nc" not in _CACHE:
        _CACHE["nc"] = _build_bass()
    return _CACHE["nc"]


def kernel(x, W_attn, b_attn, W_proj, b_proj):
    global LAST_RESULTS
    from concourse import bass_utils
    import ml_dtypes

    bf = ml_dtypes.bfloat16
    x = np.asarray(x, dtype=np.float32)
    W_attn = np.asarray(W_attn, dtype=np.float32)
    b_attn = np.asarray(b_attn, dtype=np.float32)
    W_proj = np.asarray(W_proj, dtype=np.float32)
    b_proj = np.asarray(b_proj, dtype=np.float32)

    in_maps = []
    for g in range(NCORES):
        b, hh = divmod(g, 2)
        cols = slice(hh * CPC, (hh + 1) * CPC)
        wg_g = np.ascontiguousarray(np.concatenate(
            [W_attn[:, cols], W_attn[:, C:][:, cols], W_attn[:, 2 * C:][:, cols]],
            axis=1,
        ).astype(bf))
        bg_g = np.ascontiguousarray(np.concatenate(
            [b_attn[cols], b_attn[C:][cols], b_attn[2 * C:][cols]]
        ))
        wp_g = np.ascontiguousarray(W_proj[cols, :].astype(bf))
        xt_g = np.ascontiguousarray(x[b].T.astype(bf))
        in_maps.append({"xt": xt_g, "wg": wg_g, "bg": bg_g, "wp": wp_g})

    nc = _get_nc()
    res = bass_utils.run_bass_kernel_spmd(nc, in_maps, core_ids=list(range(NCORES)))
    LAST_RESULTS = res

    out = np.empty((B, T, C), dtype=np.float32)
    for b in range(B):
        acc = res.results[2 * b]["outp"].astype(np.float64)
        acc += res.results[2 * b + 1]["outp"]
        acc += b_proj
        out[b] = acc.astype(np.float32)
    return out


# revision 11
# speedup vs baseline: 1.2894x; 1.0192x over previous
"""Causal self-attention (B=4, T=2048, C=1024, H=16, D=64) on 8 trn2 NeuronCores.

Sharding: (batch, head-half). Core g owns batch b=g//2 and head-half
hh=g%2 -> heads [8*hh, 8*hh+8) = 4 head-pairs, 512 channels. Each core
computes out_partial[2048, 1024] = y_half @ W_proj[rows of its half];
the host sums the two partials per batch (row-parallel W_proj) + b_proj.
vs. pure head-parallel sharding this cuts per-core DMA 4x (x in, out out)
at identical PE cycle count.

All matmul operands are bf16: the PE streams bf16 at 1 cyc/elem vs ~2 for
fp32. PSUM accumulation stays fp32; the softmax normalizer chain stays fp32.

Device structure per core (S^T = K @ Q^T formulation):
  - S^T tiles are [k_tok, q_tok]: exp(S)*mask and P^T @ V need no on-chip
    transposes of P.
  - V^T tiles ("va", layout [1 | V_A^T] [1 | V_B^T]) are produced by the
    DMA xbar transpose, not the PE; the leading ones column makes the PV
    matmul emit the softmax normalizer l[q] on PSUM partition 0.
  - 1/l is computed on 32 DVE lanes via the 32x32 stream-transpose trick,
    then broadcast across partitions by GpSimd partition_broadcast (no PE
    involvement in normalization at all).
  - kt-loop is software-pipelined depth 1: S(kt+1) is issued before PV(kt)
    so the PE streams S while ScalarE works exp(kt).
  - QKV projection / V-transposes / output projection of neighboring tiles
    are interleaved between attention q-tiles to keep the PE dense.
"""

import numpy as np

B, T, C, H, D = 4, 2048, 1024, 16, 64
NCORES = 8
HPC = 8                  # heads per core
NHP = 4                  # head-pairs per core
CPC = HPC * D            # 512 channels per core
NC_CHUNKS = C // 128     # 8 contraction chunks of X^T
GC = 3 * CPC             # 1536 qkv-projection output cols per core
QW = 512                 # q-tile width (moving dim)
KW = 128                 # k-tile width (S^T partition dim)
NTT = T // QW            # 4 token tiles
SCALE = 1.0 / np.sqrt(D)

_CACHE = {}
LAST_RESULTS = None      # test harness reads exec_time_ns from here


def _build_bass():
    import concourse.bass as bass
    import concourse.mybir as mybir
    import concourse.tile as tile
    from concourse import bacc
    from concourse.masks import make_upper_triangular

    f32 = mybir.dt.float32
    bf16 = mybir.dt.bfloat16
    Exp = mybir.ActivationFunctionType.Exp

    nc = bacc.Bacc()
    xt = nc.dram_tensor("xt", [C, T], bf16, kind="ExternalInput")
    wg = nc.dram_tensor("wg", [C, GC], bf16, kind="ExternalInput")
    bg = nc.dram_tensor("bg", [GC], f32, kind="ExternalInput")
    wp = nc.dram_tensor("wp", [CPC, C], bf16, kind="ExternalInput")
    outp = nc.dram_tensor("outp", [T, C], bf16, kind="ExternalOutput")

    with tile.TileContext(nc) as tc:
        with (
            tc.tile_pool(name="const", bufs=1) as cpool,
            tc.tile_pool(name="sb", bufs=2) as sb,
            tc.tile_pool(name="ps", bufs=2, space="PSUM") as ps,
        ):
            # ---- weights: wg on the gpsimd DMA ring, rest on sync, so the
            # first QKV matmuls have their operands ASAP ----
            wg_sb = []
            for ci in range(NC_CHUNKS):
                t_ = cpool.tile([128, GC], bf16, tag=f"wg{ci}", name="wgt")
                wg_sb.append(t_)
            for grp in range(3):
                for ci in range(NC_CHUNKS):
                    nc.gpsimd.dma_start(
                        out=wg_sb[ci][:, grp * CPC:(grp + 1) * CPC],
                        in_=wg[ci * 128:(ci + 1) * 128, grp * CPC:(grp + 1) * CPC],
                    )
            # bias chunk o = grp*4 + hp lives in bias_sb[:, o]
            bias_sb = cpool.tile([128, 12], f32, tag="bias")
            nc.sync.dma_start(out=bias_sb, in_=bg.rearrange("(o p) -> p o", p=128))

            # ---- constants ----
            scratch = cpool.tile([128, 128], f32, tag="scratch")
            from concourse.masks import make_identity
            make_identity(nc, scratch)
            identity = cpool.tile([128, 128], bf16, tag="ident")
            nc.vector.tensor_copy(identity, scratch)
            # mask[k, q] = 1.0 where q >= k else 0 (upper triangular incl diag)
            scratch2 = cpool.tile([128, 128], f32, tag="scratch2")
            make_upper_triangular(nc, scratch2, val=1.0, diag=True)
            mask = cpool.tile([128, 128], bf16, tag="mask")
            nc.vector.tensor_copy(mask, scratch2)
            # broadcast mask over the head axis (free-dim stride 0)
            mask2 = bass.AP(
                tensor=mask.tensor, offset=mask.offset,
                ap=[mask.ap[0], [0, 2], mask.ap[1]],
            )

            # ---- per-core persistent state ----
            # qkv_sb[grp][hp]: [128, T] bf16, rows 0:64 head 2hp, 64:128 head 2hp+1
            qkv_sb = [
                [cpool.tile([128, T], bf16, tag=f"{nm}{hp}", name=f"{nm}{hp}")
                 for hp in range(NHP)]
                for nm in ("q", "k", "v")
            ]
            vaug = [[None] * (T // KW) for _ in range(NHP)]

            def qkv_toktile(tt):
                """QKV projection for tokens [tt*QW, (tt+1)*QW)."""
                sl = slice(tt * QW, (tt + 1) * QW)
                xts = []
                for ci in range(NC_CHUNKS):
                    xtile = sb.tile([128, QW], bf16, tag="xt", bufs=16, name="xtile")
                    nc.sync.dma_start(out=xtile, in_=xt[ci * 128:(ci + 1) * 128, sl])
                    xts.append(xtile)
                for o in range(12):
                    grp, hp = divmod(o, NHP)
                    pq = ps.tile([128, QW], f32, tag="mm", name="pq")
                    for ci in range(NC_CHUNKS):
                        nc.tensor.matmul(
                            pq, wg_sb[ci][:, o * 128:(o + 1) * 128], xts[ci],
                            start=(ci == 0), stop=(ci == NC_CHUNKS - 1),
                        )
                    # bias-add doubles as the PSUM->SBUF+cast copy
                    nc.vector.tensor_scalar_add(
                        out=qkv_sb[grp][hp][:, sl], in0=pq,
                        scalar1=bias_sb[:, o:o + 1],
                    )

            def vtrans_toktile(tt):
                """V^T -> [V_A^T | 1 | V_B^T | 1] tiles via PE transpose."""
                for hp in range(NHP):
                    vt = qkv_sb[2][hp]
                    for kt in range(tt * 4, tt * 4 + 4):
                        ptr = ps.tile([128, 128], bf16, tag="mm",
                                      padded_shape=[128, 1024], name="ptr")
                        nc.tensor.transpose(
                            ptr, vt[:, kt * KW:(kt + 1) * KW], identity
                        )
                        va = cpool.tile([128, 2, D + 1], bf16, tag=f"va{hp}_{kt}",
                                        name="va")
                        nc.gpsimd.memset(va[:, :, D:D + 1], 1.0)
                        nc.vector.tensor_copy(
                            va[:, :, 0:D],
                            ptr[:, 0:2 * D].rearrange("p (h x) -> p h x", x=D),
                        )
                        vaug[hp][kt] = va

            def attention_qtile(qt, hp, yts):
                qt_sb, kt_sb = qkv_sb[0][hp], qkv_sb[1][hp]
                # y2 rows: 0..63 = y^T, 64 = l (from the ones column);
                # 96 partitions so the 32x32 l-transpose can read PSUM rows
                # 64:96 directly (same bytes/partition -> same 2 banks)
                y2 = ps.tile([96, 2, QW], f32, tag="y", bufs=1, name="y2")
                nkt = (qt + 1) * (QW // KW)
                kdiag = qt * (QW // KW)      # first diagonal k-tile

                def s_mm(kt):
                    diag = kt >= kdiag
                    qoff = (kt - kdiag) * KW if diag else 0
                    w = QW - qoff
                    qsl = slice(qt * QW + qoff, (qt + 1) * QW)
                    ksl = slice(kt * KW, (kt + 1) * KW)
                    st = ps.tile([128, 2, QW], f32, tag="st", name="st")
                    nc.tensor.matmul(
                        st[:, 0, 0:w], kt_sb[0:64, ksl], qt_sb[0:64, qsl]
                    )
                    nc.tensor.matmul(
                        st[:, 1, 0:w], kt_sb[64:128, ksl], qt_sb[64:128, qsl]
                    )
                    p = sb.tile([128, 2, QW], bf16, tag="p", bufs=4, name="p")
                    nc.scalar.activation(
                        p[:, :, 0:w], st[:, :, 0:w], Exp, scale=SCALE
                    )
                    if diag:
                        nc.vector.tensor_mul(p[:, :, 0:KW], p[:, :, 0:KW], mask2)
                    return (p, qoff, w, kt)

                def pv_mm(t):
                    p, qoff, w, kt = t
                    va = vaug[hp][kt]
                    nc.tensor.matmul(
                        y2[0:D + 1, 0, qoff:QW], va[:, 0, :], p[:, 0, 0:w],
                        start=(kt == 0), stop=(kt == nkt - 1),
                    )
                    nc.tensor.matmul(
                        y2[0:D + 1, 1, qoff:QW], va[:, 1, :], p[:, 1, 0:w],
                        start=(kt == 0), stop=(kt == nkt - 1),
                    )

                # depth-1 pipeline: S(kt+1) issues before PV(kt) so the PE
                # streams S while ScalarE runs exp(kt)
                prev = s_mm(0)
                for kt in range(1, nkt):
                    cur = s_mm(kt)
                    pv_mm(prev)
                    prev = cur
                pv_mm(prev)

                # ---- normalize: y^T * broadcast(1/l) -> yt_sb [128, QW] ----
                # l-transpose reads y2 PSUM rows 64:96 directly while the y
                # rows copy to SBUF in parallel (shorter serial chain)
                ystage = sb.tile([128, 2, QW], f32, tag="ystage", name="ystage")
                nc.vector.tensor_copy(ystage[0:D, :, :], y2[0:D, :, :])
                lrow = y2.rearrange("p h q -> p (h q)")
                # cross-base stream transpose lands the chain on partitions
                # 0:32 so partition_broadcast (which reads the tile's
                # partition 0) sees 1/l on row 0 of rcf
                lt = sb.tile([32, 2 * QW], f32, tag="lt", name="lt")
                nc.vector.transpose(lt[0:32, :], lrow[D:D + 32, :])
                rt = sb.tile([32, 2 * QW], f32, tag="rt", name="rt")
                lt_v = lt[0:32, :].rearrange("p (j c) -> p j c", c=32)
                rt_v = rt[0:32, :].rearrange("p (j c) -> p j c", c=32)
                nc.vector.reciprocal(rt_v[:, :, 0:1], lt_v[:, :, 0:1])
                rcf = sb.tile([32, 2 * QW], f32, tag="rcf", name="rcf")
                nc.vector.transpose(rcf[0:32, :], rt[0:32, :])
                # broadcast 1/l across 64 partitions on GpSimd (PE-free)
                bc = sb.tile([64, 2, QW], f32, tag="bc", name="bc")
                nc.gpsimd.partition_broadcast(
                    bc.rearrange("p h q -> p (h q)"), rcf[0:1, :]
                )
                yt_sb = sb.tile([128, QW], bf16, tag="yt", bufs=8, name="yt_sb")
                nc.vector.tensor_mul(yt_sb[0:64, :], ystage[0:D, 0, :], bc[:, 0, :])
                nc.vector.tensor_mul(yt_sb[64:128, :], ystage[0:D, 1, :], bc[:, 1, :])
                yts[hp] = yt_sb

            def proj_qtile(qt, yts):
                for m in range(QW // 128):
                    osb = sb.tile([128, C], bf16, tag="osb", bufs=3, name="osb")
                    for n in range(2):
                        pp = ps.tile([128, 512], f32, tag="mm", name="pp")
                        for hp in range(NHP):
                            nc.tensor.matmul(
                                pp, yts[hp][:, m * 128:(m + 1) * 128],
                                wp_sb[hp][:, n * 512:(n + 1) * 512],
                                start=(hp == 0), stop=(hp == NHP - 1),
                            )
                        if n == 0:
                            nc.scalar.copy(osb[:, 0:512], pp)
                        else:
                            nc.vector.tensor_copy(osb[:, 512:1024], pp)
                    row0 = qt * QW + m * 128
                    eng = nc.gpsimd if m % 2 == 0 else nc.sync
                    eng.dma_start(out=outp[row0:row0 + 128, :], in_=osb)

            # ---- software-pipelined schedule ----
            qkv_toktile(0)
            # wp only needed by the first proj (~40us in): DMA it after the
            # startup-critical loads
            wp_sb = []
            for hp in range(NHP):
                t_ = cpool.tile([128, C], bf16, tag=f"wp{hp}", name="wpt")
                nc.sync.dma_start(out=t_, in_=wp[hp * 128:(hp + 1) * 128, :])
                wp_sb.append(t_)
            vtrans_toktile(0)
            pending = []
            for qt in range(NTT):
                yts = {}
                for hp in range(NHP):
                    attention_qtile(qt, hp, yts)
                    if hp == 0 and qt + 1 < NTT:
                        qkv_toktile(qt + 1)
                    if hp == 1 and qt + 1 < NTT:
                        vtrans_toktile(qt + 1)
                    if hp == 2 and pending:
                        proj_qtile(*pending.pop())
                pending.append((qt, yts))
            proj_qtile(*pending.pop())

    nc.finalize()
    return nc


def _get_nc():
    if "nc" not in _CACHE:
        _CACHE["nc"] = _build_bass()
    return _CACHE["nc"]


def kernel(x, W_attn, b_attn, W_proj, b_proj):
    global LAST_RESULTS
    from concourse import bass_utils
    import ml_dtypes

    bf = ml_dtypes.bfloat16
    x = np.asarray(x, dtype=np.float32)
    W_attn = np.asarray(W_attn, dtype=np.float32)
    b_attn = np.asarray(b_attn, dtype=np.float32)
    W_proj = np.asarray(W_proj, dtype=np.float32)
    b_proj = np.asarray(b_proj, dtype=np.float32)

    in_maps = []
    for g in range(NCORES):
        b, hh = divmod(g, 2)
        cols = slice(hh * CPC, (hh + 1) * CPC)
        wg_g = np.ascontiguousarray(np.concatenate(
            [W_attn[:, cols], W_attn[:, C:][:, cols], W_attn[:, 2 * C:][:, cols]],
            axis=1,
        ).astype(bf))
        bg_g = np.ascontiguousarray(np.concatenate(
            [b_attn[cols], b_attn[C:][cols], b_attn[2 * C:][cols]]
        ))
        wp_g = np.ascontiguousarray(W_proj[cols, :].astype(bf))
        xt_g = np.ascontiguousarray(x[b].T.astype(bf))
        in_maps.append({"xt": xt_g, "wg": wg_g, "bg": bg_g, "wp": wp_g})

    nc = _get_nc()
    res = bass_utils.run_bass_kernel_spmd(nc, in_maps, core_ids=list(range(NCORES)))
    LAST_RESULTS = res

    out = np.empty((B, T, C), dtype=np.float32)
    for b in range(B):
        acc = res.results[2 * b]["outp"].astype(np.float32)
        acc += res.results[2 * b + 1]["outp"].astype(np.float32)
        acc += b_proj
        out[b] = acc
    return out
